# revision 30
# baseline (speedup 1.0000x reference)
"""Trainium2 Bass kernel for nn_BERTEmbedding_65274912964883.

out[b, l, :] = token_table[seq[b, l]]
             + mean_{g in genres(seq[b, l])} genre_table[g]
             + pos_table[l]

Strategy (8 NeuronCores, SPMD, no collectives):
  - Data-parallel over batch: 256 sequences -> 32 per core (6400 tokens/core).
  - Combined bf16 table paged 4 rows/page ([25000, 640] bf16, row = 160
    elems: 128 emb + 8 gid + 1 cnt + pad). Page ids fit int16, so ONE
    dma_gather instruction fetches an entire 1536-token macro tile
    (SWDGE cost ~= 994ns + 0.34ns/index, vs ~1.4us per 128 tokens for
    indirect_dma_start, which consumes only one index per partition).
  - The 4-candidate row select (q = token & 3) runs on uint32-bitcast views:
    GpSimd does the base copy (q=0), DVE does 3 predicated overwrites with
    host-precomputed q masks.
  - genre mean = (one-hot histogram over 21 genres) @ genre_table:
    padded genre slots are remapped out of range (gid + 32*(1-mask));
    s-reduction via a log-tree of contiguous adds; normalization is one
    small DVE op that downcasts to bf16 for the PE; per-subtile PE
    transposes (base partition 0) feed K=21 bf16 matmuls.
  - token + positional terms enter the same PSUM bank via PE identity
    matmuls; the Scalar engine copies histograms and finished PSUM groups
    to SBUF.
  - positional rows come from a host-prebuilt rotated table (28 rotations).
  - Macro tiles are tapered [12, 12, 12, 12, 2] to shorten the tail.
  - Device writes output partition-major [128, N/128, D] f32; host
    un-permutes.
"""

import numpy as np
import ml_dtypes

import concourse.bacc as bacc
import concourse.mybir as mybir
import concourse.tile as tile
from concourse.bass_utils import run_bass_kernel_spmd

VOCAB = 100000
D = 128
G = 21          # genre ids are in [0, 20]
MAXG = 8
CW = 144        # selected row: 128 emb + 8 gid + 1 cnt + 7 pad (bf16)
RW = 160        # table row stride (bf16 elems); page = 4 rows = 640
PAGE = 4 * RW   # 640 bf16 elems = 1280 B
NPAGE = VOCAB // 4
B, L = 256, 200
NCORES = 8
BC = B // NCORES          # sequences per core
N = BC * L                # tokens per core (6400)
SUB = 128                 # tokens per subtile (partition dim)
NSUB = N // SUB           # 50
MACROS = [12, 12, 12, 12, 2]   # subtiles per macro tile (sum = NSUB)
NROT = 25                 # distinct values of (128*i) % 200
NROTX = 28                # extended with 3 duplicates so groups never wrap

F32 = mybir.dt.float32
BF16 = mybir.dt.bfloat16
I16 = mybir.dt.int16
U32 = mybir.dt.uint32
U8 = mybir.dt.uint8

assert sum(MACROS) == NSUB


def emit_core_kernel(tc, pidx, qmask, ptab, gtab, posrot, giota, iota8, ident,
                     out):
    """Emit the per-core kernel into TileContext `tc`.

    pidx   : DRAM [128, NSUB*8] int16 -- page ids, per macro wrapped
             (index i of macro m at [i%16, 8*i0 + i//16], replicated x8)
    qmask  : DRAM [128, 3*NSUB] uint8 -- qmask[p, (q-1)*NSUB+i] = 1 if
             (seq[i*128+p] & 3) == q else 0
    ptab   : DRAM [NPAGE, PAGE] bf16 paged combined table
    gtab   : DRAM [G, D] bf16
    posrot : DRAM [128, NROTX*D] bf16
    giota  : DRAM [128, G] bf16, each row = 0..G-1
    iota8  : DRAM [128, MAXG] bf16, each row = 0..MAXG-1
    ident  : DRAM [128, 128] bf16 identity
    out    : DRAM [128, NSUB, D] f32, out[p, i, :] = embedding of token i*128+p
    """
    nc = tc.nc
    add = mybir.AluOpType.add
    mult = mybir.AluOpType.mult

    with (
        tc.tile_pool(name="const", bufs=1) as cpool,
        tc.tile_pool(name="work", bufs=2) as wpool,
        tc.tile_pool(name="psum", bufs=2, space="PSUM") as ppool,
    ):
        # --- one-time loads; pidx first (gathers depend only on it) ---
        pidx_sb = cpool.tile([128, NSUB * 8], I16)
        nc.sync.dma_start(out=pidx_sb[:], in_=pidx)
        qmask_sb = cpool.tile([128, 3 * NSUB], U8)
        nc.sync.dma_start(out=qmask_sb[:], in_=qmask)
        gtab_sb = cpool.tile([G, D], BF16)
        nc.sync.dma_start(out=gtab_sb[:], in_=gtab)
        giota_sb = cpool.tile([128, G], BF16)
        nc.sync.dma_start(out=giota_sb[:], in_=giota)
        iota8_sb = cpool.tile([128, MAXG], BF16)
        nc.sync.dma_start(out=iota8_sb[:], in_=iota8)
        ident_sb = cpool.tile([128, 128], BF16)
        nc.sync.dma_start(out=ident_sb[:], in_=ident)
        posrot_sb = cpool.tile([128, NROTX * D], BF16)
        nc.sync.dma_start(out=posrot_sb[:], in_=posrot)

        # --- main loop over macro tiles ---
        i0 = 0  # global subtile index of the macro's first subtile
        for ksub in MACROS:
            nidx = ksub * SUB
            # one dma_gather fetches all pages for the macro
            pg_sb = wpool.tile([128, ksub * PAGE], BF16, tag="pg", bufs=3)
            nc.gpsimd.dma_gather(
                out_ap=pg_sb[:].rearrange("p (j e) -> p j e", e=PAGE),
                in_ap=ptab,
                idxs_ap=pidx_sb[:, 8 * i0:8 * (i0 + ksub)],
                num_idxs=nidx,
                num_idxs_reg=nidx,
                elem_size=PAGE,
                single_packet=False,
            )

            # 4-way row select on uint32 views: base copy (q=0) on GpSimd,
            # predicated overwrites (q=1..3) on DVE
            # sel rows keep the 160-elem stride so the written 72-u32
            # blocks never merge into a flat AP (sim/lowering shape match)
            sel_sb = wpool.tile([128, ksub * RW], BF16, tag="sel", bufs=2)
            pg_u = pg_sb[:].bitcast(U32).rearrange(
                "p (j c) -> p j c", c=PAGE // 2)
            sel_u = sel_sb[:].bitcast(U32).rearrange(
                "p (j c) -> p j c", c=RW // 2)[:, :, 0:CW // 2]
            nc.gpsimd.tensor_copy(out=sel_u, in_=pg_u[:, :, 0:CW // 2])
            for q in (1, 2, 3):
                qm = qmask_sb[:, (q - 1) * NSUB + i0:(q - 1) * NSUB + i0 + ksub]
                nc.vector.copy_predicated(
                    out=sel_u,
                    mask=qm.unsqueeze(2).broadcast_to([128, ksub, CW // 2]),
                    data=pg_u[:, :, q * (RW // 2):q * (RW // 2) + CW // 2],
                )

            sel3 = sel_sb[:].rearrange("p (j c) -> p j c", c=RW)
            gid = sel3[:, :, D:D + MAXG]                # [128, ksub, MAXG]
            cnt = sel3[:, :, D + MAXG:D + MAXG + 1]     # [128, ksub, 1]

            # rec[p, j] = 1 / count
            rec_sb = wpool.tile([128, ksub], F32, tag="rec")
            nc.vector.reciprocal(rec_sb[:], sel3[:, :, D + MAXG])

            # mask[p, j, s] = (s < count[p, j])
            mask_sb = wpool.tile([128, ksub * MAXG], BF16, tag="mask")
            m3 = mask_sb[:].rearrange("p (j s) -> p j s", s=MAXG)
            nc.vector.tensor_tensor(
                out=m3,
                in0=iota8_sb[:].unsqueeze(1).broadcast_to([128, ksub, MAXG]),
                in1=cnt.broadcast_to([128, ksub, MAXG]),
                op=mybir.AluOpType.is_lt,
            )
            # shift = 32 * (1 - mask); gidm = gid + shift
            shift_sb = wpool.tile([128, ksub * MAXG], BF16, tag="shift")
            nc.vector.tensor_scalar(
                out=shift_sb[:], in0=mask_sb[:],
                scalar1=-32.0, scalar2=32.0,
                op0=mult, op1=add,
            )
            gidm_sb = wpool.tile([128, ksub * MAXG], BF16, tag="gidm")
            nc.vector.tensor_tensor(
                out=gidm_sb[:].rearrange("p (j s) -> p j s", s=MAXG),
                in0=gid,
                in1=shift_sb[:].rearrange("p (j s) -> p j s", s=MAXG),
                op=add,
            )

            # eq[p, j, s, g] = (gidm[p, j, s] == g)
            eq_sb = wpool.tile([128, ksub * MAXG * G], BF16, tag="eq")
            e4 = eq_sb[:].rearrange("p (j s g) -> p j s g", s=MAXG, g=G)
            nc.vector.tensor_tensor(
                out=e4,
                in0=gidm_sb[:].rearrange("p (j s) -> p j s", s=MAXG)
                    .unsqueeze(3).broadcast_to([128, ksub, MAXG, G]),
                in1=giota_sb[:].unsqueeze(1).unsqueeze(2).broadcast_to(
                    [128, ksub, MAXG, G]
                ),
                op=mybir.AluOpType.is_equal,
            )

            # hist_raw[p, j, g] = sum_s eq[p, j, s, g] (log-tree, contiguous)
            t1_sb = wpool.tile([128, ksub * 4 * G], BF16, tag="tree1")
            t14 = t1_sb[:].rearrange("p (j s g) -> p j s g", s=4, g=G)
            nc.vector.tensor_tensor(
                out=t14, in0=e4[:, :, 0:4, :], in1=e4[:, :, 4:8, :], op=add)
            t2_sb = wpool.tile([128, ksub * 2 * G], BF16, tag="tree2")
            t24 = t2_sb[:].rearrange("p (j s g) -> p j s g", s=2, g=G)
            nc.vector.tensor_tensor(
                out=t24, in0=t14[:, :, 0:2, :], in1=t14[:, :, 2:4, :], op=add)
            hist_sb = wpool.tile([128, ksub * G], BF16, tag="hist")
            nc.vector.tensor_tensor(
                out=hist_sb[:].rearrange("p (j g) -> p j g", g=G),
                in0=t24[:, :, 0, :], in1=t24[:, :, 1, :], op=add)

            # hist_norm = hist_raw / count   (bf16 for the PE)
            histn_sb = wpool.tile([128, ksub * G], BF16, tag="histn")
            nc.vector.tensor_tensor(
                out=histn_sb[:].rearrange("p (j g) -> p j g", g=G),
                in0=hist_sb[:].rearrange("p (j g) -> p j g", g=G),
                in1=rec_sb[:].unsqueeze(2).broadcast_to([128, ksub, G]),
                op=mult,
            )

            # per-subtile PE transpose of the histogram (base partition 0);
            # PSUM -> SBUF copies ride the Scalar engine
            histT = []
            for j in range(ksub):
                hT_ps = ppool.tile([G, 128], BF16, tag="hT_ps", bufs=3)
                nc.tensor.transpose(
                    out=hT_ps[:],
                    in_=histn_sb[:, j * G:(j + 1) * G],
                    identity=ident_sb[:],
                )
                hT_sb = wpool.tile([G, 128], BF16, tag="hT_sb", bufs=3)
                nc.scalar.copy(out=hT_sb[:], in_=hT_ps[:])
                histT.append(hT_sb)

            out_sb = wpool.tile([128, ksub * D], F32, tag="outsb", bufs=3)
            for j0 in range(0, ksub, 4):
                ng = min(4, ksub - j0)
                gm_ps = ppool.tile([128, ng * D], F32, tag="gm_ps", bufs=3)
                # token + positional terms via identity matmuls; genre last
                r0 = (i0 + j0) % NROT
                nc.tensor.matmul(
                    out=gm_ps[:],
                    lhsT=ident_sb[:],
                    rhs=sel3[:, j0:j0 + ng, 0:D],
                    start=True, stop=False,
                    skip_group_check=True,
                )
                nc.tensor.matmul(
                    out=gm_ps[:],
                    lhsT=ident_sb[:],
                    rhs=posrot_sb[:, r0 * D:(r0 + ng) * D],
                    start=False, stop=False,
                    skip_group_check=True,
                )
                for k in range(ng):
                    nc.tensor.matmul(
                        out=gm_ps[:, k * D:(k + 1) * D],
                        lhsT=histT[j0 + k][:],
                        rhs=gtab_sb[:],
                        start=False, stop=True,
                        skip_group_check=True,
                    )
                oslice = out_sb[:, j0 * D:(j0 + ng) * D]
                nc.scalar.copy(out=oslice, in_=gm_ps[:])
                # store per group
                nc.sync.dma_start(
                    out=out[:, i0 + j0:i0 + j0 + ng, :],
                    in_=out_sb[:, j0 * D:(j0 + ng) * D]
                        .rearrange("p (j d) -> p j d", d=D),
                )
            i0 += ksub


def build_nc():
    nc = bacc.Bacc("TRN2", target_bir_lowering=False, debug=False)
    pidx = nc.dram_tensor("pidx", [128, NSUB * 8], I16, kind="ExternalInput").ap()
    qmask = nc.dram_tensor(
        "qmask", [128, 3 * NSUB], U8, kind="ExternalInput").ap()
    ptab = nc.dram_tensor("ptab", [NPAGE, PAGE], BF16, kind="ExternalInput").ap()
    gtab = nc.dram_tensor("gtab", [G, D], BF16, kind="ExternalInput").ap()
    posrot = nc.dram_tensor(
        "posrot", [128, NROTX * D], BF16, kind="ExternalInput").ap()
    giota = nc.dram_tensor("giota", [128, G], BF16, kind="ExternalInput").ap()
    iota8 = nc.dram_tensor("iota8", [128, MAXG], BF16, kind="ExternalInput").ap()
    ident = nc.dram_tensor("ident", [128, 128], BF16, kind="ExternalInput").ap()
    out = nc.dram_tensor("out", [128, NSUB, D], F32, kind="ExternalOutput").ap()

    with tile.TileContext(nc) as tc:
        emit_core_kernel(tc, pidx, qmask, ptab, gtab, posrot, giota, iota8,
                         ident, out)
    nc.compile()
    return nc


_NC_CACHE = None


def _get_nc():
    global _NC_CACHE
    if _NC_CACHE is None:
        _NC_CACHE = build_nc()
    return _NC_CACHE


def make_ptab(token_table, token_genre_ids, genre_counts):
    rows = np.zeros((VOCAB, RW), dtype=ml_dtypes.bfloat16)
    rows[:, 0:D] = np.asarray(token_table, dtype=np.float32).astype(
        ml_dtypes.bfloat16)
    rows[:, D:D + MAXG] = np.asarray(
        token_genre_ids, dtype=np.float32).astype(ml_dtypes.bfloat16)
    rows[:, D + MAXG] = np.asarray(
        genre_counts, dtype=np.float32).astype(ml_dtypes.bfloat16)
    return np.ascontiguousarray(rows.reshape(NPAGE, PAGE))


def make_posrot(pos_table):
    pos = np.asarray(pos_table, dtype=np.float32)
    pr = np.zeros((128, NROTX * D), dtype=np.float32)
    p = np.arange(128)
    for r in range(NROTX):
        pr[:, r * D:(r + 1) * D] = pos[(128 * r + p) % L, :]
    return pr.astype(ml_dtypes.bfloat16)


def make_pidx_qmask(seq_core):
    """seq_core: [N] int64/int32 token ids for one core."""
    t = seq_core.reshape(NSUB, 128).astype(np.int64)
    pages = (t >> 2).astype(np.int16)      # [NSUB, 128]
    q = (t & 3).astype(np.int64)           # [NSUB, 128]

    pidx = np.zeros((16, NSUB * 8), dtype=np.int16)
    i0 = 0
    for ksub in MACROS:
        flat = pages[i0:i0 + ksub, :].reshape(-1)   # i = j*128 + p
        pidx[:, 8 * i0:8 * (i0 + ksub)] = flat.reshape(ksub * 8, 16).T
        i0 += ksub
    pidx = np.ascontiguousarray(np.tile(pidx, (8, 1)))  # replicate to 128

    qmask = np.zeros((128, 3 * NSUB), dtype=np.uint8)
    qT = q.T  # [128, NSUB]
    for qq in (1, 2, 3):
        qmask[:, (qq - 1) * NSUB:qq * NSUB] = (qT == qq).astype(np.uint8)
    return pidx, qmask


def prep_host_inputs(sequence, token_table, genre_table, pos_table,
                     token_genre_ids, genre_counts):
    """Host-side sharding / layout prep. Returns in_maps for the 8 cores."""
    seq = np.ascontiguousarray(np.asarray(sequence).astype(np.int64)).reshape(B, L)
    ptab = make_ptab(token_table, token_genre_ids, genre_counts)
    gtab = np.asarray(genre_table, dtype=np.float32).astype(ml_dtypes.bfloat16)
    posrot = make_posrot(pos_table)

    giota = np.broadcast_to(
        np.arange(G, dtype=np.float32), (128, G)).astype(ml_dtypes.bfloat16)
    iota8 = np.broadcast_to(
        np.arange(MAXG, dtype=np.float32), (128, MAXG)).astype(
        ml_dtypes.bfloat16)
    ident = np.eye(128, dtype=np.float32).astype(ml_dtypes.bfloat16)

    in_maps = []
    for c in range(NCORES):
        seq_core = seq[c * BC:(c + 1) * BC].reshape(N)
        pidx, qmask = make_pidx_qmask(seq_core)
        in_maps.append({
            "pidx": pidx,
            "qmask": qmask,
            "ptab": ptab,
            "gtab": gtab,
            "posrot": posrot,
            "giota": giota,
            "iota8": iota8,
            "ident": ident,
        })
    return in_maps


def postprocess(results):
    """Un-permute per-core outputs and concatenate to [B, L, D]."""
    outs = []
    for c in range(NCORES):
        o = results[c]["out"]  # [128, NSUB, D]
        outs.append(np.ascontiguousarray(o.transpose(1, 0, 2)).reshape(BC, L, D))
    return np.concatenate(outs, axis=0)


def kernel(sequence, token_table, genre_table, pos_table, token_genre_ids,
           genre_counts):
    nc = _get_nc()
    in_maps = prep_host_inputs(sequence, token_table, genre_table, pos_table,
                               token_genre_ids, genre_counts)
    res = run_bass_kernel_spmd(nc, in_maps, core_ids=list(range(NCORES)))
    return postprocess(res.results)


# revision 31
# speedup vs baseline: 1.0630x; 1.0630x over previous
"""Trainium2 Bass kernel for nn_BERTEmbedding_65274912964883.

out[b, l, :] = token_table[seq[b, l]]
             + mean_{g in genres(seq[b, l])} genre_table[g]
             + pos_table[l]

Strategy (8 NeuronCores, SPMD, no collectives):
  - Data-parallel over batch: 256 sequences -> 32 per core (6400 tokens/core).
  - Combined bf16 table paged 4 rows/page ([25000, 640] bf16, row = 160
    elems: 128 emb + 8 gid + 1 cnt + pad). Page ids fit int16, so ONE
    dma_gather instruction fetches an entire 1536-token macro tile
    (SWDGE cost ~= 994ns + 0.34ns/index, vs ~1.4us per 128 tokens for
    indirect_dma_start, which consumes only one index per partition).
  - The 4-candidate row select (q = token & 3) runs on uint32-bitcast views:
    GpSimd does the base copy (q=0), DVE does 3 predicated overwrites with
    host-precomputed q masks.
  - genre mean = (one-hot histogram over 21 genres) @ genre_table:
    padded genre slots are remapped out of range (gid + 32*(1-mask));
    s-reduction via a log-tree of contiguous adds; normalization is one
    small DVE op that downcasts to bf16 for the PE; per-subtile PE
    transposes (base partition 0) feed K=21 bf16 matmuls.
  - token + positional terms enter the same PSUM bank via PE identity
    matmuls; the Scalar engine copies histograms and finished PSUM groups
    to SBUF.
  - positional rows come from a host-prebuilt rotated table (28 rotations).
  - Macro tiles are tapered [12, 12, 12, 12, 2] to shorten the tail.
  - Device writes output partition-major [128, N/128, D] f32; host
    un-permutes.
"""

import numpy as np
import ml_dtypes

import concourse.bacc as bacc
import concourse.mybir as mybir
import concourse.tile as tile
from concourse.bass_utils import run_bass_kernel_spmd

VOCAB = 100000
D = 128
G = 21          # genre ids are in [0, 20]
MAXG = 8
CW = 144        # selected row: 128 emb + 8 gid + 1 cnt + 7 pad (bf16)
RW = 160        # table row stride (bf16 elems); page = 4 rows = 640
PAGE = 4 * RW   # 640 bf16 elems = 1280 B
NPAGE = VOCAB // 4
B, L = 256, 200
NCORES = 8
BC = B // NCORES          # sequences per core
N = BC * L                # tokens per core (6400)
SUB = 128                 # tokens per subtile (partition dim)
NSUB = N // SUB           # 50
MACROS = [12, 12, 12, 12, 2]   # subtiles per macro tile (sum = NSUB)
NROT = 25                 # distinct values of (128*i) % 200
NROTX = 28                # extended with 3 duplicates so groups never wrap

F32 = mybir.dt.float32
BF16 = mybir.dt.bfloat16
I16 = mybir.dt.int16
U32 = mybir.dt.uint32
U8 = mybir.dt.uint8

assert sum(MACROS) == NSUB


def emit_core_kernel(tc, pidx, qmask, ptab, gtab, posrot, giota, iota8, ident,
                     out):
    """Emit the per-core kernel into TileContext `tc`.

    pidx   : DRAM [128, NSUB*8] int16 -- page ids, per macro wrapped
             (index i of macro m at [i%16, 8*i0 + i//16], replicated x8)
    qmask  : DRAM [128, 3*NSUB] uint8 -- qmask[p, (q-1)*NSUB+i] = 1 if
             (seq[i*128+p] & 3) == q else 0
    ptab   : DRAM [NPAGE, PAGE] bf16 paged combined table
    gtab   : DRAM [G, D] bf16
    posrot : DRAM [128, NROTX*D] bf16
    giota  : DRAM [128, G] bf16, each row = 0..G-1
    iota8  : DRAM [128, MAXG] bf16, each row = 0..MAXG-1
    ident  : DRAM [128, 128] bf16 identity
    out    : DRAM [128, NSUB, D] f32, out[p, i, :] = embedding of token i*128+p
    """
    nc = tc.nc
    add = mybir.AluOpType.add
    mult = mybir.AluOpType.mult

    with (
        tc.tile_pool(name="const", bufs=1) as cpool,
        tc.tile_pool(name="work", bufs=2) as wpool,
        tc.tile_pool(name="psum", bufs=2, space="PSUM") as ppool,
    ):
        # --- one-time loads; pidx first (gathers depend only on it) ---
        pidx_sb = cpool.tile([128, NSUB * 8], I16)
        nc.sync.dma_start(out=pidx_sb[:], in_=pidx)
        qmask_sb = cpool.tile([128, 3 * NSUB], U8)
        nc.sync.dma_start(out=qmask_sb[:], in_=qmask)
        gtab_sb = cpool.tile([G, D], BF16)
        nc.sync.dma_start(out=gtab_sb[:], in_=gtab)
        giota_sb = cpool.tile([128, G], BF16)
        nc.sync.dma_start(out=giota_sb[:], in_=giota)
        iota8_sb = cpool.tile([128, MAXG], BF16)
        nc.sync.dma_start(out=iota8_sb[:], in_=iota8)
        ident_sb = cpool.tile([128, 128], BF16)
        nc.sync.dma_start(out=ident_sb[:], in_=ident)
        posrot_sb = cpool.tile([128, NROTX * D], BF16)
        nc.sync.dma_start(out=posrot_sb[:], in_=posrot)

        # --- main loop over macro tiles ---
        i0 = 0  # global subtile index of the macro's first subtile
        for ksub in MACROS:
            nidx = ksub * SUB
            # one dma_gather fetches all pages for the macro
            pg_sb = wpool.tile([128, ksub * PAGE], BF16, tag="pg", bufs=3)
            nc.gpsimd.dma_gather(
                out_ap=pg_sb[:].rearrange("p (j e) -> p j e", e=PAGE),
                in_ap=ptab,
                idxs_ap=pidx_sb[:, 8 * i0:8 * (i0 + ksub)],
                num_idxs=nidx,
                num_idxs_reg=nidx,
                elem_size=PAGE,
                single_packet=False,
            )

            # 4-way row select on uint32 views: base copy (q=0) on GpSimd,
            # predicated overwrites (q=1..3) on DVE
            # sel rows keep the 160-elem stride so the written 72-u32
            # blocks never merge into a flat AP (sim/lowering shape match)
            sel_sb = wpool.tile([128, ksub * RW], BF16, tag="sel", bufs=2)
            pg_u = pg_sb[:].bitcast(U32).rearrange(
                "p (j c) -> p j c", c=PAGE // 2)
            sel_u = sel_sb[:].bitcast(U32).rearrange(
                "p (j c) -> p j c", c=RW // 2)[:, :, 0:CW // 2]
            nc.vector.tensor_copy(out=sel_u, in_=pg_u[:, :, 0:CW // 2])
            for q in (1, 2, 3):
                qm = qmask_sb[:, (q - 1) * NSUB + i0:(q - 1) * NSUB + i0 + ksub]
                nc.vector.copy_predicated(
                    out=sel_u,
                    mask=qm.unsqueeze(2).broadcast_to([128, ksub, CW // 2]),
                    data=pg_u[:, :, q * (RW // 2):q * (RW // 2) + CW // 2],
                )

            sel3 = sel_sb[:].rearrange("p (j c) -> p j c", c=RW)
            gid = sel3[:, :, D:D + MAXG]                # [128, ksub, MAXG]
            cnt = sel3[:, :, D + MAXG:D + MAXG + 1]     # [128, ksub, 1]

            # rec[p, j] = 1 / count
            rec_sb = wpool.tile([128, ksub], F32, tag="rec")
            nc.vector.reciprocal(rec_sb[:], sel3[:, :, D + MAXG])

            # mask[p, j, s] = (s < count[p, j])
            mask_sb = wpool.tile([128, ksub * MAXG], BF16, tag="mask")
            m3 = mask_sb[:].rearrange("p (j s) -> p j s", s=MAXG)
            nc.vector.tensor_tensor(
                out=m3,
                in0=iota8_sb[:].unsqueeze(1).broadcast_to([128, ksub, MAXG]),
                in1=cnt.broadcast_to([128, ksub, MAXG]),
                op=mybir.AluOpType.is_lt,
            )
            # shift = 32 * (1 - mask); gidm = gid + shift
            shift_sb = wpool.tile([128, ksub * MAXG], BF16, tag="shift")
            nc.vector.tensor_scalar(
                out=shift_sb[:], in0=mask_sb[:],
                scalar1=-32.0, scalar2=32.0,
                op0=mult, op1=add,
            )
            gidm_sb = wpool.tile([128, ksub * MAXG], BF16, tag="gidm")
            nc.vector.tensor_tensor(
                out=gidm_sb[:].rearrange("p (j s) -> p j s", s=MAXG),
                in0=gid,
                in1=shift_sb[:].rearrange("p (j s) -> p j s", s=MAXG),
                op=add,
            )

            # eq[p, j, s, g] = (gidm[p, j, s] == g)
            eq_sb = wpool.tile([128, ksub * MAXG * G], BF16, tag="eq")
            e4 = eq_sb[:].rearrange("p (j s g) -> p j s g", s=MAXG, g=G)
            nc.vector.tensor_tensor(
                out=e4,
                in0=gidm_sb[:].rearrange("p (j s) -> p j s", s=MAXG)
                    .unsqueeze(3).broadcast_to([128, ksub, MAXG, G]),
                in1=giota_sb[:].unsqueeze(1).unsqueeze(2).broadcast_to(
                    [128, ksub, MAXG, G]
                ),
                op=mybir.AluOpType.is_equal,
            )

            # hist_raw[p, j, g] = sum_s eq[p, j, s, g] (log-tree, contiguous)
            t1_sb = wpool.tile([128, ksub * 4 * G], BF16, tag="tree1")
            t14 = t1_sb[:].rearrange("p (j s g) -> p j s g", s=4, g=G)
            nc.vector.tensor_tensor(
                out=t14, in0=e4[:, :, 0:4, :], in1=e4[:, :, 4:8, :], op=add)
            t2_sb = wpool.tile([128, ksub * 2 * G], BF16, tag="tree2")
            t24 = t2_sb[:].rearrange("p (j s g) -> p j s g", s=2, g=G)
            nc.vector.tensor_tensor(
                out=t24, in0=t14[:, :, 0:2, :], in1=t14[:, :, 2:4, :], op=add)
            hist_sb = wpool.tile([128, ksub * G], BF16, tag="hist")
            nc.vector.tensor_tensor(
                out=hist_sb[:].rearrange("p (j g) -> p j g", g=G),
                in0=t24[:, :, 0, :], in1=t24[:, :, 1, :], op=add)

            # hist_norm = hist_raw / count   (bf16 for the PE)
            histn_sb = wpool.tile([128, ksub * G], BF16, tag="histn")
            nc.vector.tensor_tensor(
                out=histn_sb[:].rearrange("p (j g) -> p j g", g=G),
                in0=hist_sb[:].rearrange("p (j g) -> p j g", g=G),
                in1=rec_sb[:].unsqueeze(2).broadcast_to([128, ksub, G]),
                op=mult,
            )

            # per-subtile PE transpose of the histogram (base partition 0);
            # PSUM -> SBUF copies ride the Scalar engine
            histT = []
            for j in range(ksub):
                hT_ps = ppool.tile([G, 128], BF16, tag="hT_ps", bufs=3)
                nc.tensor.transpose(
                    out=hT_ps[:],
                    in_=histn_sb[:, j * G:(j + 1) * G],
                    identity=ident_sb[:],
                )
                hT_sb = wpool.tile([G, 128], BF16, tag="hT_sb", bufs=3)
                nc.scalar.copy(out=hT_sb[:], in_=hT_ps[:])
                histT.append(hT_sb)

            out_sb = wpool.tile([128, ksub * D], F32, tag="outsb", bufs=3)
            for j0 in range(0, ksub, 4):
                ng = min(4, ksub - j0)
                gm_ps = ppool.tile([128, ng * D], F32, tag="gm_ps", bufs=3)
                # token + positional terms via identity matmuls; genre last
                r0 = (i0 + j0) % NROT
                nc.tensor.matmul(
                    out=gm_ps[:],
                    lhsT=ident_sb[:],
                    rhs=sel3[:, j0:j0 + ng, 0:D],
                    start=True, stop=False,
                    skip_group_check=True,
                )
                nc.tensor.matmul(
                    out=gm_ps[:],
                    lhsT=ident_sb[:],
                    rhs=posrot_sb[:, r0 * D:(r0 + ng) * D],
                    start=False, stop=False,
                    skip_group_check=True,
                )
                for k in range(ng):
                    nc.tensor.matmul(
                        out=gm_ps[:, k * D:(k + 1) * D],
                        lhsT=histT[j0 + k][:],
                        rhs=gtab_sb[:],
                        start=False, stop=True,
                        skip_group_check=True,
                    )
                oslice = out_sb[:, j0 * D:(j0 + ng) * D]
                nc.scalar.copy(out=oslice, in_=gm_ps[:])
                # store per group
                nc.sync.dma_start(
                    out=out[:, i0 + j0:i0 + j0 + ng, :],
                    in_=out_sb[:, j0 * D:(j0 + ng) * D]
                        .rearrange("p (j d) -> p j d", d=D),
                )
            i0 += ksub


def build_nc():
    nc = bacc.Bacc("TRN2", target_bir_lowering=False, debug=False)
    pidx = nc.dram_tensor("pidx", [128, NSUB * 8], I16, kind="ExternalInput").ap()
    qmask = nc.dram_tensor(
        "qmask", [128, 3 * NSUB], U8, kind="ExternalInput").ap()
    ptab = nc.dram_tensor("ptab", [NPAGE, PAGE], BF16, kind="ExternalInput").ap()
    gtab = nc.dram_tensor("gtab", [G, D], BF16, kind="ExternalInput").ap()
    posrot = nc.dram_tensor(
        "posrot", [128, NROTX * D], BF16, kind="ExternalInput").ap()
    giota = nc.dram_tensor("giota", [128, G], BF16, kind="ExternalInput").ap()
    iota8 = nc.dram_tensor("iota8", [128, MAXG], BF16, kind="ExternalInput").ap()
    ident = nc.dram_tensor("ident", [128, 128], BF16, kind="ExternalInput").ap()
    out = nc.dram_tensor("out", [128, NSUB, D], F32, kind="ExternalOutput").ap()

    with tile.TileContext(nc) as tc:
        emit_core_kernel(tc, pidx, qmask, ptab, gtab, posrot, giota, iota8,
                         ident, out)
    nc.compile()
    return nc


_NC_CACHE = None


def _get_nc():
    global _NC_CACHE
    if _NC_CACHE is None:
        _NC_CACHE = build_nc()
    return _NC_CACHE


def make_ptab(token_table, token_genre_ids, genre_counts):
    rows = np.zeros((VOCAB, RW), dtype=ml_dtypes.bfloat16)
    rows[:, 0:D] = np.asarray(token_table, dtype=np.float32).astype(
        ml_dtypes.bfloat16)
    rows[:, D:D + MAXG] = np.asarray(
        token_genre_ids, dtype=np.float32).astype(ml_dtypes.bfloat16)
    rows[:, D + MAXG] = np.asarray(
        genre_counts, dtype=np.float32).astype(ml_dtypes.bfloat16)
    return np.ascontiguousarray(rows.reshape(NPAGE, PAGE))


def make_posrot(pos_table):
    pos = np.asarray(pos_table, dtype=np.float32)
    pr = np.zeros((128, NROTX * D), dtype=np.float32)
    p = np.arange(128)
    for r in range(NROTX):
        pr[:, r * D:(r + 1) * D] = pos[(128 * r + p) % L, :]
    return pr.astype(ml_dtypes.bfloat16)


def make_pidx_qmask(seq_core):
    """seq_core: [N] int64/int32 token ids for one core."""
    t = seq_core.reshape(NSUB, 128).astype(np.int64)
    pages = (t >> 2).astype(np.int16)      # [NSUB, 128]
    q = (t & 3).astype(np.int64)           # [NSUB, 128]

    pidx = np.zeros((16, NSUB * 8), dtype=np.int16)
    i0 = 0
    for ksub in MACROS:
        flat = pages[i0:i0 + ksub, :].reshape(-1)   # i = j*128 + p
        pidx[:, 8 * i0:8 * (i0 + ksub)] = flat.reshape(ksub * 8, 16).T
        i0 += ksub
    pidx = np.ascontiguousarray(np.tile(pidx, (8, 1)))  # replicate to 128

    qmask = np.zeros((128, 3 * NSUB), dtype=np.uint8)
    qT = q.T  # [128, NSUB]
    for qq in (1, 2, 3):
        qmask[:, (qq - 1) * NSUB:qq * NSUB] = (qT == qq).astype(np.uint8)
    return pidx, qmask


def prep_host_inputs(sequence, token_table, genre_table, pos_table,
                     token_genre_ids, genre_counts):
    """Host-side sharding / layout prep. Returns in_maps for the 8 cores."""
    seq = np.ascontiguousarray(np.asarray(sequence).astype(np.int64)).reshape(B, L)
    ptab = make_ptab(token_table, token_genre_ids, genre_counts)
    gtab = np.asarray(genre_table, dtype=np.float32).astype(ml_dtypes.bfloat16)
    posrot = make_posrot(pos_table)

    giota = np.broadcast_to(
        np.arange(G, dtype=np.float32), (128, G)).astype(ml_dtypes.bfloat16)
    iota8 = np.broadcast_to(
        np.arange(MAXG, dtype=np.float32), (128, MAXG)).astype(
        ml_dtypes.bfloat16)
    ident = np.eye(128, dtype=np.float32).astype(ml_dtypes.bfloat16)

    in_maps = []
    for c in range(NCORES):
        seq_core = seq[c * BC:(c + 1) * BC].reshape(N)
        pidx, qmask = make_pidx_qmask(seq_core)
        in_maps.append({
            "pidx": pidx,
            "qmask": qmask,
            "ptab": ptab,
            "gtab": gtab,
            "posrot": posrot,
            "giota": giota,
            "iota8": iota8,
            "ident": ident,
        })
    return in_maps


def postprocess(results):
    """Un-permute per-core outputs and concatenate to [B, L, D]."""
    outs = []
    for c in range(NCORES):
        o = results[c]["out"]  # [128, NSUB, D]
        outs.append(np.ascontiguousarray(o.transpose(1, 0, 2)).reshape(BC, L, D))
    return np.concatenate(outs, axis=0)


def kernel(sequence, token_table, genre_table, pos_table, token_genre_ids,
           genre_counts):
    nc = _get_nc()
    in_maps = prep_host_inputs(sequence, token_table, genre_table, pos_table,
                               token_genre_ids, genre_counts)
    res = run_bass_kernel_spmd(nc, in_maps, core_ids=list(range(NCORES)))
    return postprocess(res.results)
